# revision 25
# baseline (speedup 1.0000x reference)
"""BatchRankingLoss on TRN2 — PE hinge matmuls + chunked relu-accum +
host-masked band correction.

loss = 2/N * sum_{t_i - t_j > 0.1} relu(1 + o_i - o_j)   (pair symmetry;
groups host-sorted by t so active pairs have j < i, bounded by
c[g,i] = #{j: bf16(t_j) < bf16(t_i - 0.1)}).

Layout: 64 groups/core, partition p = (g, par); slice s covers i in
{8s..8s+7} (par + 4 column-interleaved slots e); free col = 4j + e.

Device:
  PE:  D = (1 + o_i) - o_j over j < J_s via K=68 matmuls per slice
       (64 group-indicator rows + 4 o-value rows; indicator arrives
       pre-replicated via DMA; extents > 512 split into 2 pieces),
       FFD-packed PSUM chunks.
  ACT/DVE: per chunk relu(D) with accum_out -> S1 (includes inactive
       cols c <= j < J_s).
  DVE: correction relu+accum over OB -> S2, where host packs the
       inactive band cells [L_s, J_s) as bf16(b - o_j), active -> -100.
Host: loss = 2 * (S1 - S2) / N.
"""

import os
import numpy as np
from contextlib import ExitStack

import concourse.bacc as bacc
import concourse.mybir as mybir
import concourse.tile as tile
from concourse.bass_utils import run_bass_kernel_spmd
import ml_dtypes

BF16 = ml_dtypes.bfloat16

N_CORES = 8
D = 256
G_REAL = 511
G_PAD = 512
GPC = 64
NS = 32                   # slices per core, 8 i-values each
KDIM = 68
N_PAIRS = G_REAL * D * (D - 1)
MARGIN = 2
PSUM_F32 = 512
ACHUNK = 2048
_CACHED = {}


def _pack_chunks(exts, cap=PSUM_F32):
    """First-fit-decreasing bin packing (slices in a chunk need not be
    consecutive — matmuls write at arbitrary offsets, relu reads the
    whole chunk)."""
    order = sorted((s for s in range(len(exts)) if exts[s] > 0),
                   key=lambda s: -exts[s])
    chunks, tots = [], []
    for s in order:
        e = exts[s]
        for i in range(len(chunks)):
            if tots[i] + e <= cap:
                chunks[i].append(s)
                tots[i] += e
                break
        else:
            chunks.append([s])
            tots.append(e)
    return chunks


def _build_program(JS, LS, BW, repeat=1, loop=0):
    nc = bacc.Bacc("TRN2", target_bir_lowering=False, debug=False,
                   num_devices=N_CORES)
    f32 = mybir.dt.float32
    bf16 = mybir.dt.bfloat16
    A = mybir.AluOpType

    ES = [4 * j for j in JS]
    # split slice extents into <=512-col matmul pieces, FFD-pack pieces
    pieces = []
    for s in range(NS):
        c0 = 0
        while c0 < ES[s]:
            pieces.append((s, c0, min(ES[s], c0 + 512)))
            c0 += 512
    chunks = _pack_chunks([p[2] - p[1] for p in pieces])
    chunks = [[pieces[i] for i in ch] for ch in chunks]
    NCH = len(chunks)

    # static engine split for the relu chunks: ACT ~800ns per 512-col
    # chunk, DVE ~650ns; DVE also carries the band (~2us)
    relu_eng = {}
    la = float(os.environ.get("BRL_LA0", "-4000"))
    lv = 2000.0 + BW * 0.45
    for ci, ch in enumerate(chunks):
        cols = sum(c1 - c0 for _, c0, c1 in ch)
        ca = cols * 0.833 + 370.0
        cv = cols * 1.042 + 120.0
        if la + ca <= lv + cv:
            relu_eng[ci] = "act"
            la += ca
        else:
            relu_eng[ci] = "dve"
            lv += cv

    bchunks = []
    b0 = 0
    while b0 < BW:
        bchunks.append((b0, min(BW, b0 + ACHUNK)))
        b0 += ACHUNK
    assert NCH <= 48 and len(bchunks) <= 16

    ind_d = nc.dram_tensor("ind", [GPC, NS * 128], bf16, kind="ExternalInput")
    wvo_d = nc.dram_tensor("wv_o", [4, NS * 128], bf16, kind="ExternalInput")
    ro_d = nc.dram_tensor("ro", [KDIM, 1024], bf16, kind="ExternalInput")
    ob_d = nc.dram_tensor("ob", [128, max(BW, 1)], bf16, kind="ExternalInput")
    macc_d = nc.dram_tensor("m_acc", [128, 64], f32, kind="ExternalOutput")

    with ExitStack() as ctx:
        tc = ctx.enter_context(tile.TileContext(nc, num_cores=N_CORES))
        consts = ctx.enter_context(tc.tile_pool(name="consts", bufs=1))
        psd_pool = ctx.enter_context(tc.tile_pool(name="psd", bufs=6, space="PSUM"))
        rd_pool = ctx.enter_context(tc.tile_pool(name="rd", bufs=6))
        scrap_pool = ctx.enter_context(tc.tile_pool(name="scrap", bufs=2))

        w = consts.tile([KDIM, NS * 128], bf16)
        ro = consts.tile([KDIM, 1024], bf16)
        ob = consts.tile([128, max(BW, 1)], bf16)
        macc = consts.tile([128, 64], f32)

        nc.sync.dma_start(w[0:GPC, :], ind_d[:])
        nc.sync.dma_start(ro[:], ro_d[:])
        nc.sync.dma_start(w[GPC:KDIM, :], wvo_d[:])
        if BW > 0:
            half = (BW // 2) & ~3
            nc.scalar.dma_start(ob[:, 0:half], ob_d[:, 0:half])
            nc.gpsimd.dma_start(ob[:, half:BW], ob_d[:, half:BW])
        nc.vector.memset(macc[:], 0.0)

        loop_cm = tc.For_i(0, loop, 1) if loop else None
        if loop_cm is not None:
            loop_cm.__enter__()
        for _rep in range(repeat):
            for ci, ch in enumerate(chunks):
                ext = sum(c1 - c0 for _, c0, c1 in ch)
                psd = psd_pool.tile([128, PSUM_F32], f32, tag="d")
                off = 0
                for s, c0, c1 in ch:
                    nc.tensor.matmul(
                        psd[:, off:off + (c1 - c0)],
                        lhsT=w[0:KDIM, s * 128:(s + 1) * 128],
                        rhs=ro[0:KDIM, c0:c1],
                        start=True, stop=True,
                    )
                    off += c1 - c0
                rd = rd_pool.tile([128, PSUM_F32], bf16, tag="rd")
                acc = macc[:, ci:ci + 1]
                if relu_eng[ci] == "act":
                    nc.scalar.activation(
                        rd[:, 0:ext], psd[:, 0:ext],
                        mybir.ActivationFunctionType.Relu, accum_out=acc)
                else:
                    nc.vector.tensor_scalar(
                        out=rd[:, 0:ext], in0=psd[:, 0:ext],
                        scalar1=0.0, scalar2=0.0, op0=A.max, op1=A.add,
                        accum_out=acc)
            for k, (b0, b1) in enumerate(bchunks):
                scrap = scrap_pool.tile([128, ACHUNK], bf16, tag="sc")
                nc.vector.tensor_scalar(
                    out=scrap[:, 0:b1 - b0], in0=ob[:, b0:b1],
                    scalar1=0.0, scalar2=0.0, op0=A.max, op1=A.add,
                    accum_out=macc[:, 48 + k:48 + k + 1])
        if loop_cm is not None:
            loop_cm.__exit__(None, None, None)
        nc.sync.dma_start(macc_d[:], macc[:])

    nc.compile()
    return nc


def _host_prep(t_all, o_all):
    t_g = np.zeros((G_PAD, D), dtype=np.float32)
    o_g = np.zeros((G_PAD, D), dtype=np.float32)
    t_g[:G_REAL] = t_all.reshape(G_REAL, D)
    o_g[:G_REAL] = o_all.reshape(G_REAL, D)
    idx = np.argsort(t_g, axis=1)
    t_g = np.take_along_axis(t_g, idx, axis=1)
    o_g = np.take_along_axis(o_g, idx, axis=1)

    tbf = t_g.astype(BF16).astype(np.float32)
    tbv = (t_g - np.float32(0.1)).astype(BF16).astype(np.float32)
    c = np.empty((G_PAD, D), dtype=np.int64)
    for g in range(G_REAL):
        c[g] = np.searchsorted(tbf[g], tbv[g], side="left")
    c[G_REAL:] = 0

    cr = c[:G_REAL].reshape(G_REAL, NS, 8)       # [g, s, 8i]
    cmax = cr.max(axis=(0, 2))
    cmin = cr.min(axis=(0, 2))
    JS, LS = [], []
    for s in range(NS):
        if int(cmax[s]) == 0:
            JS.append(0)
            LS.append(0)
            continue
        j = min(D, int(cmax[s]) + 1)
        l = max(0, min(int(cmin[s]) - MARGIN, j))
        JS.append(j)
        LS.append(l)
    return t_g, o_g, c, JS, LS


def _prep_core_inputs(t_g, o_g, c, JS, LS, core):
    g0 = core * GPC
    o_c = o_g[g0:g0 + GPC]
    is_last = core == N_CORES - 1

    ov = (np.float32(1.0) + o_c)
    if is_last:
        ov[GPC - 1, :] = -1000.0
    # value rows (o): [e, s*128 + (2g+par)] = ov[g, 8s+2e+par]
    a = ov.astype(BF16).astype(np.float32).reshape(GPC, NS, 4, 2)
    wv_o = np.ascontiguousarray(
        a.transpose(2, 1, 0, 3).reshape(4, NS * 128)).astype(BF16)

    ind = (np.arange(128)[None, :] // 2 ==
           np.arange(GPC)[:, None]).astype(BF16)
    ind = np.ascontiguousarray(np.tile(ind, (1, NS)))

    ro = np.zeros((KDIM, 1024), dtype=BF16)
    mo = (-o_c).astype(BF16)
    for e in range(4):
        ro[:GPC, e::4] = mo
        ro[GPC + e, e::4] = BF16(1.0)

    # band OB: per live slice, cols (e, j) for j in [L, J): inactive ->
    # bf16(b - o_j), active -> -100. Packed [128, sum 2W].
    c_c = c[g0:g0 + GPC]
    bcv = a  # [g, s, e, par] = bf16-rounded 1 + o_i (or -1000 pad)
    obs = []
    for s in range(NS):
        if JS[s] == 0:
            continue
        L, J = LS[s], JS[s]
        W = J - L
        j_idx = np.arange(L, J)
        b_slab = bcv[:, s, :, :].reshape(GPC, 4, 2, 1)       # [g, e, par, 1]
        o_slab = o_c[:, None, L:J].astype(BF16).astype(np.float32)
        o_slab = o_slab.reshape(GPC, 1, 1, W)
        vals = (b_slab - o_slab).astype(np.float32)          # [g, e, par, W]
        ii = (8 * s + 2 * np.arange(4)[None, :, None] +
              np.arange(2)[None, None, :])                   # [1, e, par]
        cc = np.take_along_axis(
            c_c[:, :], np.broadcast_to(ii, (GPC, 4, 2)).reshape(GPC, 8),
            axis=1).reshape(GPC, 4, 2, 1)
        inactive = j_idx[None, None, None, :] >= cc
        vals = np.where(inactive, vals, np.float32(-100.0))
        # -> [p = 2g+par, e*W + w]
        vals = vals.transpose(0, 2, 1, 3).reshape(128, 4 * W)
        obs.append(vals)
    ob = (np.concatenate(obs, axis=1) if obs
          else np.zeros((128, 1), np.float32)).astype(BF16)
    return {"ind": ind, "wv_o": wv_o, "ro": ro,
            "ob": np.ascontiguousarray(ob)}


def combine(res):
    total = np.float64(0.0)
    for cc in range(N_CORES):
        m = res.results[cc]["m_acc"].astype(np.float64)
        total += m[:, :48].sum() - m[:, 48:].sum()
    return 2.0 * total / float(N_PAIRS)


def kernel(input, gdt_ts):
    o_all = np.asarray(input).reshape(-1)[: G_REAL * D].astype(np.float32, copy=False)
    t_all = np.asarray(gdt_ts).reshape(-1)[: G_REAL * D].astype(np.float32, copy=False)

    t_g, o_g, c, JS, LS = _host_prep(t_all, o_all)
    in_maps = [_prep_core_inputs(t_g, o_g, c, JS, LS, cc) for cc in range(N_CORES)]
    BW = in_maps[0]["ob"].shape[1]
    for m in in_maps:
        assert m["ob"].shape[1] == BW

    key = (tuple(JS), tuple(LS), BW)
    if _CACHED.get("key") != key:
        _CACHED.update(key=key, nc=_build_program(JS, LS, BW))
    res = run_bass_kernel_spmd(_CACHED["nc"], in_maps, list(range(N_CORES)))
    return np.array([combine(res)], dtype=np.float32)
